# revision 1
# baseline (speedup 1.0000x reference)
"""CapsuleLayer (dynamic routing) Trainium2 kernel — 8 NeuronCores, I-sharded.

Reference computation (per problem):
  u_hat = einsum('oidc,bic->boid', W, x)           # B=64 O=32 I=2048 D=32 C=16
  b_ij = 0; 3 routing iterations of:
    c = softmax_O(b_ij); s = einsum('boi,boid->bod', c, u_hat); v = squash(s)
    b_ij += einsum('boid,bod->boi', u_hat, v)      # (first 2 iters)
  return v                                          # [B, O, D]

Sharding: I=2048 split 8 ways (IL=256/core).  W-slice (16.75MB) stays resident
in SBUF as bf16; u_hat is recomputed on the PE per routing pass (cheaper than
HBM round-trips).  Per-iteration cross-core traffic is a single 256KB
AllReduce of the s partial sums.

Per-core layouts (p = SBUF partition index):
  w_sd [p=(i8*16+c), f=(oct*1024 + o*32+d)]  : rhs of u_hat matmul, bf16
  x_bd [p=(i8*16+c), f=((q*32+oct)*128 + b16*8+i8')] : block-diag lhsT, bf16
  xt   [p=(i8*16+c), f=(oct*64 + b)]         : lhsT of s0 matmul, bf16
  u_hat psum/sbuf tiles [p=(b16*8+i8), f=(o*32+d)] per (q, oct)
  agreement/softmax     [p=(b16*8+i8), f=(oct*128 + q*32 + o)]
  s psum  [p=(32q + o2*16 + b16), f=(op*64 + o2'*32 + d)]  (o = 2*op + o2)
"""

import sys

sys.path.insert(0, "/opt/trn_rl_repo")

import numpy as np
import ml_dtypes

import concourse.bass as bass
import concourse.mybir as mybir
from concourse import bacc
from concourse.tile import TileContext
from concourse.bass_utils import run_bass_kernel_spmd

BF16 = mybir.dt.bfloat16
F32 = mybir.dt.float32
AF = mybir.ActivationFunctionType
ALU = mybir.AluOpType

B, O, I, D, C = 64, 32, 2048, 32, 16
NCORES = 8
IL = I // NCORES          # 256 i's per core
NOCT = IL // 8            # 32 octets of 8 i's
EPS = 1e-9

_CACHE = {}


def _ap(t, poff, pcnt, dims, foff=0):
    """AP with partition slice [poff, poff+pcnt) and free dims [[step, count], ...]
    (steps in elements) at free-element offset foff."""
    base = t if isinstance(t, bass.AP) else t.ap()
    pitch = base.ap[0][0]
    return bass.AP(base.tensor, base.offset + poff * pitch + foff,
                   [[pitch, pcnt], *dims])


def build_program(niters=2, skip_setup=False):
    """niters: number of routing iterations (2 = the real kernel).
    skip_setup=True builds a near-trivial program (dispatch-floor probe)."""
    nc = bacc.Bacc("TRN2", target_bir_lowering=False, debug=False,
                   num_devices=NCORES)

    # ---- DRAM I/O ----
    w_sd_d = nc.dram_tensor("w_sd", [128, NOCT * 1024], BF16, kind="ExternalInput")
    x_bd_d = nc.dram_tensor("x_bd", [128, 4 * NOCT * 128], BF16, kind="ExternalInput")
    xt_d = nc.dram_tensor("xt", [128, NOCT * 64], BF16, kind="ExternalInput")
    mask_d = nc.dram_tensor("mask_bd", [128, 32], BF16, kind="ExternalInput")
    out_d = nc.dram_tensor("out", [B, O * D], F32, kind="ExternalOutput")

    v_dram = nc.dram_tensor("v_bounce", [B, O * D], BF16)
    ncc = niters + 1
    cc_in = [nc.dram_tensor(f"cc_in{k}", [B, O * D], F32) for k in range(ncc)]
    cc_out = [nc.dram_tensor(f"cc_out{k}", [B, O * D], F32, addr_space="Shared")
              for k in range(ncc)]

    # ---- persistent SBUF ----
    w_sd = nc.alloc_sbuf_tensor("w_sd_sb", [128, NOCT * 1024], BF16)
    x_bd = nc.alloc_sbuf_tensor("x_bd_sb", [128, 4 * NOCT * 128], BF16)
    xt = nc.alloc_sbuf_tensor("xt_sb", [128, NOCT * 64], BF16)
    mask = nc.alloc_sbuf_tensor("mask_sb", [128, 32], BF16)
    b_sb = nc.alloc_sbuf_tensor("b_sb", [128, NOCT * 128], F32)
    vrep = nc.alloc_sbuf_tensor("vrep_sb", [128, 4 * 1024], BF16)
    s_sb = nc.alloc_sbuf_tensor("s_sb", [128, 1024], F32)
    sq_sb = nc.alloc_sbuf_tensor("sq_sb", [B, 1024], F32)
    v32_sb = nc.alloc_sbuf_tensor("v32_sb", [B, 1024], F32)
    v16_sb = nc.alloc_sbuf_tensor("v16_sb", [B, 1024], BF16)

    # s accumulation psum: 2 banks, rows 32q+(o2*16+b16), cols op*64+o2'*32+d
    s_ps = nc.alloc_psum_tensor("s_ps", [128, 1024], F32)
    s0_ps = nc.alloc_psum_tensor("s0_ps", [B, 1024], F32)

    if skip_setup:
        with TileContext(nc) as tc:
            with tc.tile_pool(name="triv", bufs=1) as tp:
                t = tp.tile([B, 1024], F32)
                nc.sync.dma_start(t[:], cc_in[0][:])
                nc.sync.dma_start(out_d[:], t[:])
        nc.compile()
        return nc

    with TileContext(nc) as tc:
        with (
            tc.tile_pool(name="pu", bufs=4, space="PSUM") as pupool,
            tc.tile_pool(name="work", bufs=2) as wpool,
            tc.tile_pool(name="small", bufs=4) as spool,
        ):
            # ---- load persistent inputs ----
            nc.sync.dma_start(w_sd[:], w_sd_d[:])
            nc.sync.dma_start(x_bd[:], x_bd_d[:])
            nc.sync.dma_start(xt[:], xt_d[:])
            nc.sync.dma_start(mask[:], mask_d[:])
            nc.vector.memset(b_sb[:], 0.0)

            # ================= s0 = (1/32) * sum_i u_hat ====================
            for half in range(2):
                for t in range(NOCT):
                    nc.tensor.matmul(
                        s0_ps[:, half * 512:(half + 1) * 512],
                        xt[:, t * 64:(t + 1) * 64],
                        w_sd[:, t * 1024 + half * 512: t * 1024 + (half + 1) * 512],
                        start=(t == 0), stop=(t == NOCT - 1),
                    )
            # copy with 1/32 scale, to sbuf, then allreduce
            nc.scalar.activation(sq_sb[:], s0_ps[:], AF.Copy, scale=1.0 / O)
            nc.sync.dma_start(cc_in[0][:], sq_sb[:])
            nc.gpsimd.collective_compute(
                "AllReduce", ALU.add, replica_groups=[list(range(NCORES))],
                ins=[cc_in[0].ap()], outs=[cc_out[0].ap()],
            )
            nc.sync.dma_start(sq_sb[:], cc_out[0][:])

            def squash_and_v(k):
                """sq_sb holds s [B, (o,d)] fp32 (already allreduced).
                Produces v32_sb; for k<2 also v16/v_dram/vrep."""
                sq2 = spool.tile([B, 1024], F32, tag="sq2")
                nrm = spool.tile([B, 32], F32, tag="nrm")
                den = spool.tile([B, 32], F32, tag="den")
                rcp = spool.tile([B, 32], F32, tag="rcp")
                fac = spool.tile([B, 32], F32, tag="fac")
                sqt = spool.tile([B, 32], F32, tag="sqt")
                nc.scalar.activation(sq2[:], sq_sb[:], AF.Square)
                nc.vector.reduce_sum(
                    nrm[:], _ap(sq2, 0, B, [[32, 32], [1, 32]]),
                    axis=mybir.AxisListType.X)
                # den = (1+nrm)*sqrt(nrm+eps)
                nc.scalar.activation(sqt[:], nrm[:], AF.Sqrt)
                nc.scalar.add(den[:], nrm[:], 1.0)
                nc.vector.tensor_mul(den[:], den[:], sqt[:])
                nc.vector.reciprocal(rcp[:], den[:])
                nc.vector.tensor_mul(fac[:], nrm[:], rcp[:])
                # v = s * fac (broadcast fac over d)
                nc.vector.scalar_tensor_tensor(
                    v32_sb[:], sq_sb[:], 1.0,
                    _ap(fac, 0, B, [[1, 32], [0, 32]]),
                    op0=ALU.mult, op1=ALU.mult)
                if k < niters:
                    nc.vector.tensor_copy(v16_sb[:], v32_sb[:])
                    nc.sync.dma_start(v_dram[:], v16_sb[:])
                    for q in range(4):
                        # vrep[p=(b16,i8), q*1024 + od] = v[b, od]
                        nc.sync.dma_start(
                            _ap(vrep, 0, 128, [[1, 1024]], foff=q * 1024),
                            bass.AP(v_dram, q * 16 * 1024,
                                    [[1024, 16], [0, 8], [1, 1024]]),
                        )

            squash_and_v(0)

            # ================= routing iterations ===========================
            for it in range(1, 1 + niters):
                for oct_ in range(NOCT):
                    U_tiles = [None] * 4
                    for q in range(4):
                        pa = pupool.tile([128, 512], F32, tag="pu")
                        pb = pupool.tile([128, 512], F32, tag="pu")
                        lhs = x_bd[:, (q * NOCT + oct_) * 128:
                                   (q * NOCT + oct_ + 1) * 128]
                        nc.tensor.matmul(pa[:], lhs,
                                         w_sd[:, oct_ * 1024: oct_ * 1024 + 512],
                                         start=True, stop=True)
                        nc.tensor.matmul(pb[:], lhs,
                                         w_sd[:, oct_ * 1024 + 512: oct_ * 1024 + 1024],
                                         start=True, stop=True)
                        U = wpool.tile([128, 1024], BF16, tag=f"U{q}")
                        U_tiles[q] = U
                        nc.scalar.activation(U[:, 0:512], pa[:], AF.Copy)
                        nc.vector.tensor_copy(U[:, 512:1024], pb[:])
                        # agreement partial: tmp = U * vrep ; tree-reduce over d
                        tmp = wpool.tile([128, 1024], BF16, tag="tmp")
                        nc.vector.tensor_mul(
                            tmp[:], U[:], vrep[:, q * 1024:(q + 1) * 1024])
                        t16 = wpool.tile([128, 512], BF16, tag="t16")
                        nc.vector.tensor_add(
                            _ap(t16, 0, 128, [[16, 32], [1, 16]]),
                            _ap(tmp, 0, 128, [[32, 32], [1, 16]]),
                            _ap(tmp, 0, 128, [[32, 32], [1, 16]], foff=16))
                        t8 = wpool.tile([128, 256], BF16, tag="t8")
                        nc.vector.tensor_add(
                            _ap(t8, 0, 128, [[8, 32], [1, 8]]),
                            _ap(t16, 0, 128, [[16, 32], [1, 8]]),
                            _ap(t16, 0, 128, [[16, 32], [1, 8]], foff=8))
                        t4 = wpool.tile([128, 128], BF16, tag="t4")
                        nc.vector.tensor_add(
                            _ap(t4, 0, 128, [[4, 32], [1, 4]]),
                            _ap(t8, 0, 128, [[8, 32], [1, 4]]),
                            _ap(t8, 0, 128, [[8, 32], [1, 4]], foff=4))
                        t2 = wpool.tile([128, 64], BF16, tag="t2")
                        nc.vector.tensor_add(
                            _ap(t2, 0, 128, [[2, 32], [1, 2]]),
                            _ap(t4, 0, 128, [[4, 32], [1, 2]]),
                            _ap(t4, 0, 128, [[4, 32], [1, 2]], foff=2))
                        t1 = wpool.tile([128, 32], F32, tag="t1")
                        nc.vector.tensor_add(
                            t1[:],
                            _ap(t2, 0, 128, [[2, 32]]),
                            _ap(t2, 0, 128, [[2, 32]], foff=1))
                        bsl = b_sb[:, oct_ * 128 + q * 32: oct_ * 128 + (q + 1) * 32]
                        nc.vector.tensor_add(bsl, bsl, t1[:])

                    # softmax over o for this octet (all 4 q at once)
                    bsl = _ap(b_sb, 0, 128, [[32, 4], [1, 32]], foff=oct_ * 128)
                    mx = spool.tile([128, 4], F32, tag="mx")
                    nc.vector.reduce_max(mx[:], bsl, axis=mybir.AxisListType.X)
                    bs = spool.tile([128, 128], F32, tag="bs")
                    nc.vector.tensor_sub(
                        bs[:], _ap(b_sb, 0, 128, [[1, 128]], foff=oct_ * 128),
                        _ap(mx, 0, 128, [[1, 4], [0, 32]]))
                    ex = spool.tile([128, 128], BF16, tag="ex")
                    nc.scalar.activation(ex[:], bs[:], AF.Exp)
                    sm = spool.tile([128, 4], F32, tag="sm")
                    nc.vector.reduce_sum(
                        sm[:], _ap(ex, 0, 128, [[32, 4], [1, 32]]),
                        axis=mybir.AxisListType.X)
                    rc = spool.tile([128, 4], F32, tag="rc")
                    nc.vector.reciprocal(rc[:], sm[:])
                    co = spool.tile([128, 128], BF16, tag="co")
                    nc.vector.tensor_mul(
                        co[:], ex[:], _ap(rc, 0, 128, [[1, 4], [0, 32]]))

                    for q in range(4):
                        cbd = wpool.tile([128, 512], BF16, tag=f"cbd{q}")
                        # cbd[p, (op,o2,b')] = mask[p, (o2,b')] * co[p, (q, 2op+o2)]
                        nc.vector.tensor_mul(
                            cbd[:],
                            _ap(mask, 0, 128, [[0, 16], [16, 2], [1, 16]]),
                            _ap(co, 0, 128, [[2, 16], [1, 2], [0, 16]],
                                foff=q * 32))
                        U = U_tiles[q]
                        for op in range(16):
                            nc.tensor.matmul(
                                _ap(s_ps, 32 * q, 32, [[1, 64]], foff=op * 64),
                                cbd[:, op * 32:(op + 1) * 32],
                                U[:, op * 64:(op + 1) * 64],
                                start=(oct_ == 0 and op % 8 == 0),
                                stop=(oct_ == NOCT - 1 and op % 8 == 7),
                                tile_position=(0, 32 * q),
                            )

                # extract s from psum -> s_sb, dma to cc, allreduce
                for q in range(4):
                    nc.vector.tensor_copy(
                        _ap(s_sb, 32 * q, 32, [[1, 1024]]),
                        _ap(s_ps, 32 * q, 32, [[1, 1024]]))
                k = it
                for q in range(4):
                    for o2 in range(2):
                        nc.sync.dma_start(
                            bass.AP(cc_in[k], q * 16 * 1024 + o2 * 32,
                                    [[1024, 16], [64, 16], [1, 32]]),
                            _ap(s_sb, 32 * q + 16 * o2, 16, [[64, 16], [1, 32]],
                                foff=o2 * 32))
                nc.gpsimd.collective_compute(
                    "AllReduce", ALU.add, replica_groups=[list(range(NCORES))],
                    ins=[cc_in[k].ap()], outs=[cc_out[k].ap()],
                )
                nc.sync.dma_start(sq_sb[:], cc_out[k][:])
                squash_and_v(k)

            # final v -> out
            nc.sync.dma_start(out_d[:], v32_sb[:])

    nc.compile()
    return nc


def prep_inputs(x, W):
    """Full [B,I,C] x and [O,I,D,C] W -> per-core input maps."""
    x = np.asarray(x, np.float32)
    W = np.asarray(W, np.float32)
    maps = []
    # identity mask for cbd: [p=(b16*8+i8), (o2,b')] = (b16 == b')
    m = (np.arange(16)[:, None, None, None] == np.arange(16)[None, None, None, :])
    mask = np.broadcast_to(m, (16, 8, 2, 16)).reshape(128, 32)
    mask = np.ascontiguousarray(mask, dtype=ml_dtypes.bfloat16)
    for c in range(NCORES):
        Wc = W[:, c * IL:(c + 1) * IL]                    # [O, IL, D, C]
        xc = x[:, c * IL:(c + 1) * IL]                    # [B, IL, C]
        w_sd = (Wc.reshape(O, NOCT, 8, D, C)
                .transpose(2, 4, 1, 0, 3)                 # [i8, c, t, o, d]
                .reshape(128, NOCT * 1024))
        xt = (xc.reshape(B, NOCT, 8, C)
              .transpose(2, 3, 1, 0)                      # [i8, c, t, b]
              .reshape(128, NOCT * 64))
        xr = (xc.reshape(4, 16, NOCT, 8, C)
              .transpose(3, 4, 0, 2, 1))                  # [i8, c, q, t, b16]
        xbd = np.zeros((8, C, 4, NOCT, 16, 8), np.float32)
        for j in range(8):
            xbd[j, :, :, :, :, j] = xr[j]
        x_bd = xbd.reshape(128, 4 * NOCT * 128)
        maps.append({
            "w_sd": np.ascontiguousarray(w_sd.astype(ml_dtypes.bfloat16)),
            "x_bd": np.ascontiguousarray(x_bd.astype(ml_dtypes.bfloat16)),
            "xt": np.ascontiguousarray(xt.astype(ml_dtypes.bfloat16)),
            "mask_bd": mask,
        })
    return maps


def kernel(x, W):
    if "nc" not in _CACHE:
        _CACHE["nc"] = build_program()
    nc = _CACHE["nc"]
    maps = prep_inputs(x, W)
    res = run_bass_kernel_spmd(nc, maps, list(range(NCORES)))
    out = res.results[0]["out"]
    return np.ascontiguousarray(out.reshape(B, O, D).astype(np.float32))

